# revision 11
# baseline (speedup 1.0000x reference)
"""Trainium2 Bass kernel: batched single-head self-attention.

Reference computation (per (b, l) pair, 20 independent blocks):
    X = x[b, l] viewed as [N=1024, D=256] (xf layout)
    out[b, l] = softmax(beta * X @ X.T, axis=-1) @ X

Device algorithm (per block):
  * Scores: S[m, n] = sum_d X^T[d, m] X^T[d, n] on the TensorEngine with
    D on partitions. S is symmetric, so the PSUM tile doubles as the
    [keys, queries] orientation the second matmul wants.
  * Softmax shift: W[m, n] = exp(beta * (S[m, n] - c_n)) with
    c_n = ||x_n||^2. The per-QUERY shift rides the score matmul as one
    extra K=1 accumulation term (lhsT = ones row, rhs = -c row).
  * Second matmul: O^T[d, n] = sum_m xfo[m, d] W[m, n] with the value
    operand xfo = [X | 1 | 0] stationary. The [1|0] chunk gives the
    softmax denominator Z_n as a separate accumulation pass.
    Normalization (divide by Z) and the [d, n] -> [n, d] flip happen on
    the host.
  * Dtypes: score operands fp16 (10-bit mantissa; measured end-to-end
    rel err ~3e-3 vs the 2e-2 gate), W tiles and values bf16 (W must be
    bf16 not fp16: exp(S - c_n) reaches e^60, past fp16 range). 16-bit
    weights let every LDWEIGHTS use the fast weight-load path -- fp32
    weight loads (~224 ns per 128x128, no FWL) were the cadence limiter
    of the fp32r version (~275 ns per 512-col matmul vs 213 ns ideal).
  * Software pipelining: the AV matmuls of key tile a are emitted after
    the score matmuls of tile a+1, so ScalarE's exp of tile a runs under
    the tile-a+1 score streams and the PE never waits on ACT.
  * The Z pass of block s is emitted between the first and second score
    tiles of block s+1 so the PSUM evacuation it depends on overlaps
    score streaming instead of stalling the PE.

Sharding: 20 blocks over 8 cores as 2 full blocks + 1 half block (512
queries) per core. The half blocks use a host-side rotation of the key
axis so every core runs the identical program (softmax is invariant to
key permutation when values are permuted identically).
"""

import numpy as np
import ml_dtypes

import concourse.tile as tile
from concourse import bacc, mybir
from concourse.bass_utils import run_bass_kernel_spmd

F32 = mybir.dt.float32
F16 = mybir.dt.float16
BF16 = mybir.dt.bfloat16

B, L, D, H, W = 4, 5, 256, 32, 32
N = H * W            # 1024 keys per block
NBLK = B * L         # 20
NCORES = 8
NFULL = 2            # full blocks per core
NSLAB = 3            # 2 full + 1 half
DF = 272             # value operand row: [x(256) | 1 | 0...] padded so bf16
                     # rows are 544 B = 17x32 B (32 B-aligned weight rows)

EXP = mybir.ActivationFunctionType.Exp


def build_program(beta: float, fast: bool = True):
    sdt = F16 if fast else F32    # score operand dtype
    wdt = BF16 if fast else F32   # W tiles / value operand dtype
    # bf16, not fp16: the UNNORMALIZED O^T rows reach ~e^60 on contested
    # softmax rows (W = exp(S - c_n) can exceed fp16 range before the
    # host-side divide by Z)
    odt = BF16 if fast else F32   # output dtype
    nc = bacc.Bacc("TRN2", target_bir_lowering=False, debug=False,
                   num_devices=NCORES)
    # Inputs are host-packed in device layout so every DMA is a plain
    # contiguous transfer with large descriptors.
    xb_in = nc.dram_tensor("xb_in", [NSLAB, 2, 128, N], sdt,
                           kind="ExternalInput")
    xf_in = nc.dram_tensor("xf_in", [NSLAB, 128, 8, DF], wdt,
                           kind="ExternalInput")
    nc_in = nc.dram_tensor("nc_in", [1, NSLAB * N], sdt, kind="ExternalInput")
    yt_out = nc.dram_tensor("yt_out", [NSLAB, 128, 2, N], odt,
                            kind="ExternalOutput")
    z_out = nc.dram_tensor("z_out", [NSLAB, N], F32, kind="ExternalOutput")

    with tile.TileContext(nc) as tc:
        _build(tc, nc, xb_in.ap(), xf_in.ap(), nc_in.ap(), yt_out.ap(),
               z_out.ap(), beta, sdt, wdt, odt)
    nc.finalize()
    return nc


def _build(tc, nc, xb_in, xf_in, nc_in, yt_out, z_out, beta, sdt, wdt, odt):
    import contextlib
    ctx = contextlib.ExitStack()
    with ctx:
        const = ctx.enter_context(tc.tile_pool(name="const", bufs=1))
        xb_pool = ctx.enter_context(tc.tile_pool(name="xb", bufs=NSLAB))
        xfo_pool = ctx.enter_context(tc.tile_pool(name="xfo", bufs=NSLAB))
        negc_pool = ctx.enter_context(tc.tile_pool(name="negc", bufs=NSLAB))
        # W tiles stay live until the Z pass at the end of the block.
        w_pool = ctx.enter_context(tc.tile_pool(name="w", bufs=10))
        ot_sb_pool = ctx.enter_context(tc.tile_pool(name="ot_sb", bufs=2))
        z_sb_pool = ctx.enter_context(tc.tile_pool(name="z_sb", bufs=2))
        # PSUM: 2 score slots x 2 banks + 4 O^T accumulator banks = 8.
        # The Z-row accumulators reuse the O^T slots (same tag) after the
        # key loop, once the block's accumulators are evacuated.
        ps_s = ctx.enter_context(tc.tile_pool(name="ps_s", bufs=2, space="PSUM"))
        ps_od = ctx.enter_context(tc.tile_pool(name="ps_od", bufs=4, space="PSUM"))

        # negcb: shift rows on partition 0, partitions 1..127 zeroed (they
        # meet zero weights in the rider, but NaN garbage x 0 = NaN).
        # Memset split across DVE and GpSimd so it finishes before the
        # first rider needs it.
        negcb = negc_pool.tile([128, NSLAB * N], sdt, tag="negc")
        half = NSLAB * N // 2
        nc.vector.memset(negcb[:, 0:half], 0.0)
        nc.gpsimd.memset(negcb[:, half:], 0.0)
        negcs = [negcb[:, s * N:(s + 1) * N] for s in range(NSLAB)]

        # Shift-rider stationary: a K=128 full-array operand with
        # partition 0 all-ones, rest zero, so lhsT.T @ negcb broadcasts
        # partition 0 of the moving operand to all 128 output rows.
        # A K=1 ones-row works too but a 1x128 (row-group-masked) matmul
        # forces a full PE drain before the next full-array LDWEIGHTS --
        # measured ~430 ns stall per rider.
        e1 = const.tile([128, 128], sdt)
        nc.gpsimd.memset(e1[:], 0.0)
        nc.gpsimd.memset(e1[0:1, :], 1.0)

        # Warm the PE clock (HAM) with throwaway matmuls that run during
        # the input-DMA window -- otherwise the first ~3.4us of real
        # matmuls run at half clock. Short 128-col streams keep the PE
        # queue shallow so the first real matmul isn't delayed.
        warm_ps = ps_od.tile([128, 512], F32, tag="od", name="warm_ps")
        for wi in range(6):
            nc.tensor.matmul(warm_ps[:, 0:128], e1[:], e1[:],
                             start=True, stop=True)

        # All input DMAs upfront, spread over queues so slab 0 lands
        # first: sync takes xb s0 chunk 0 then slabs 1-2; scalar takes
        # xb s0 chunk 1, the shift rows, then the value operands.
        xbs, xfos = [], []
        for s in range(NSLAB):
            xb = xb_pool.tile([128, 2, N], sdt, tag="xb", name=f"xb_{s}")
            xbs.append(xb)
        nc.sync.dma_start(out=xbs[0][:, 0, :], in_=xb_in[0][0])
        nc.scalar.dma_start(out=xbs[0][:, 1, :], in_=xb_in[0][1])
        for s in (1, 2):
            nc.sync.dma_start(
                out=xbs[s][:], in_=xb_in[s].rearrange("c p n -> p c n"))
        nc.scalar.dma_start(out=negcb[0:1, 0:half], in_=nc_in[:, 0:half])
        nc.scalar.dma_start(out=negcb[0:1, half:], in_=nc_in[:, half:])
        for s in range(NSLAB):
            xfo = xfo_pool.tile([128, 8, DF], wdt, tag="xfo",
                                name=f"xfo_{s}")
            nc.scalar.dma_start(out=xfo[:], in_=xf_in[s])
            xfos.append(xfo)

        def emit_scores(s, a, n_h):
            """Score matmuls + shift rider + exp for key tile a of slab s.
            Returns the W tile."""
            xb, negc = xbs[s], negcs[s]
            asl = slice(a * 128, (a + 1) * 128)
            sps = ps_s.tile([128, N], F32, tag="sps", name=f"sps_{s}_{a}")
            for c in range(2):
                for h in range(n_h):
                    hs = slice(h * 512, (h + 1) * 512)
                    nc.tensor.matmul(sps[:, hs], xb[:, c, asl], xb[:, c, hs],
                                     start=(c == 0), stop=False)
            for h in range(n_h):
                hs = slice(h * 512, (h + 1) * 512)
                nc.tensor.matmul(sps[:, hs], e1[:], negc[:, hs],
                                 start=False, stop=True)
            # W = exp(beta * S'), one ACT pass per query half so the first
            # AV matmuls only wait on their own half.
            wt = w_pool.tile([128, N], wdt, tag="w", name=f"w_{s}_{a}")
            for h in range(n_h):
                hs = slice(h * 512, (h + 1) * 512)
                nc.scalar.activation(wt[:, hs], sps[:, hs], EXP,
                                     scale=float(beta))
            return wt

        def emit_av(s, a, n_h, od, wt):
            """O^T += xfo[a].T @ W[a] (value operand stationary)."""
            xfo = xfos[s]
            for ci, csl in ((0, slice(0, 128)), (1, slice(128, 256))):
                for h in range(n_h):
                    hs = slice(h * 512, (h + 1) * 512)
                    nc.tensor.matmul(od[ci][h][:], xfo[:, a, csl],
                                     wt[:, hs],
                                     start=(a == 0), stop=(a == 7))

        # Emission order (PE program order), 1-deep pipelined:
        #   scores(s,0) scores(s,1) av(s,0) scores(s,2) av(s,1) ...
        #   scores(s,7) av(s,6) av(s,7) zpass(s) scores(s+1,0) ...
        # The Z-row accumulator lives in a score-bank slot (free after
        # exp of the block's last tile), so the Z matmuls start right
        # after av(s,7) while the O^T evacuation runs on DVE/ACT in
        # parallel; the next block's AV then finds its banks free.
        for s in range(NSLAB):
            n_q = N if s < NFULL else N // 2
            n_h = n_q // 512
            od = None
            w_tiles = []
            for a in range(8):
                wt = emit_scores(s, a, n_h)
                w_tiles.append(wt)
                if a >= 1:
                    if od is None:
                        od = [[ps_od.tile([128, 512], F32, tag="od",
                                          name=f"od_{s}_{ci}_{h}")
                               for h in range(n_h)] for ci in range(2)]
                    emit_av(s, a - 1, n_h, od, w_tiles[a - 1])
            emit_av(s, 7, n_h, od, w_tiles[7])

            # Z[n] = sum_m W[m, n] via the [1|0] chunk of xfo, into a
            # score-bank slot.
            xfo = xfos[s]
            oz = ps_s.tile([128, N], F32, tag="sps", name=f"oz_{s}")
            for a in range(8):
                for h in range(n_h):
                    hs = slice(h * 512, (h + 1) * 512)
                    nc.tensor.matmul(oz[0:2, hs], xfo[:, a, 256:258],
                                     w_tiles[a][:, hs],
                                     start=(a == 0), stop=(a == 7))

            # Evacuate O^T accumulators on DVE/ACT, overlapping the Z
            # matmuls on the PE.
            ot_sb = ot_sb_pool.tile([128, 2, N], odt, tag="ot_sb",
                                    name=f"ot_{s}")
            z_sb = z_sb_pool.tile([1, N], F32, tag="z_sb", name=f"z_{s}")
            for h in range(n_h):
                hs = slice(h * 512, (h + 1) * 512)
                nc.vector.tensor_copy(ot_sb[:, 0, hs], od[0][h][:])
                nc.scalar.copy(ot_sb[:, 1, hs], od[1][h][:])
            for h in range(n_h):
                hs = slice(h * 512, (h + 1) * 512)
                # split across DVE and ACT so the copies run in parallel
                # and release the Z bank sooner
                if h == 0:
                    nc.vector.tensor_copy(z_sb[:, hs], oz[0:1, hs])
                else:
                    nc.scalar.copy(z_sb[:, hs], oz[0:1, hs])
            nc.sync.dma_start(out=yt_out[s], in_=ot_sb[:])
            nc.sync.dma_start(out=z_out[s][:n_q].unsqueeze(0),
                              in_=z_sb[:, :n_q])


_PROG_CACHE = {}


def _get_program(beta: float, fast: bool = True):
    key = (beta, fast)
    if key not in _PROG_CACHE:
        _PROG_CACHE[key] = build_program(beta, fast)
    return _PROG_CACHE[key]


def make_in_maps(x: np.ndarray, fast: bool = True):
    """Shard the full input [B, L, D, H, W] into 8 per-core input maps."""
    sdt = np.float16 if fast else np.float32
    wdt = ml_dtypes.bfloat16 if fast else np.float32
    xt_all = np.ascontiguousarray(x.reshape(NBLK, D, N))
    in_maps = []
    for c in range(NCORES):
        half_blk = NFULL * NCORES + c // 2
        half = xt_all[half_blk]
        if c % 2 == 1:
            # rotate keys so this core's queries are columns 0..511
            half = np.concatenate([half[:, N // 2:], half[:, :N // 2]], axis=1)
        slabs = np.stack([xt_all[NFULL * c], xt_all[NFULL * c + 1], half])
        slabs16 = slabs.astype(sdt)
        # shift row from the rounded operands (any per-query shift cancels
        # exactly in O/Z; using the rounded data keeps the overflow margin)
        s32 = slabs16.astype(np.float32)
        negc = -np.einsum('sdn,sdn->sn', s32, s32)
        xf = np.zeros((NSLAB, N, DF), np.float32)
        xf[:, :, :D] = slabs.transpose(0, 2, 1)
        xf[:, :, D] = 1.0
        # pack into device layout: xb [2, 128, N], xf [128, 8, DF]
        xb_p = slabs16.reshape(NSLAB, 2, 128, N)
        xf_p = np.ascontiguousarray(
            xf.reshape(NSLAB, 8, 128, DF).transpose(0, 2, 1, 3)).astype(wdt)
        in_maps.append({"xb_in": np.ascontiguousarray(xb_p),
                        "xf_in": xf_p,
                        "nc_in": np.ascontiguousarray(
                            negc.reshape(1, NSLAB * N)).astype(sdt)})
    return in_maps


def assemble_output(results):
    """Normalize, transpose and gather per-core outputs into [B, L, N, D]."""
    out = np.empty((NBLK, N, D), np.float32)
    for c in range(NCORES):
        # yt [NSLAB, 128, 2, N]: partition p, value chunk ci -> O^T row
        # ci*128 + p
        yt = np.asarray(results[c]["yt_out"], dtype=np.float32)
        yt = yt.transpose(0, 2, 1, 3).reshape(NSLAB, 2 * 128, N)
        z = results[c]["z_out"]
        for s, blk, lo, n_q in ((0, NFULL * c, 0, N),
                                (1, NFULL * c + 1, 0, N),
                                (2, NFULL * NCORES + c // 2,
                                 (c % 2) * (N // 2), N // 2)):
            ot = yt[s, :, :n_q]                       # [D, n_q], unnormalized
            out[blk, lo:lo + n_q] = (ot / z[s, :n_q]).T
    return out.reshape(B, L, N, D)


def kernel(x, beta, _trace=False, _fast=True, _tmpdir=None):
    x = np.asarray(x, dtype=np.float32)
    assert x.shape == (B, L, D, H, W), x.shape
    beta_f = float(np.asarray(beta))
    prog = _get_program(beta_f, _fast)
    in_maps = make_in_maps(x, _fast)
    res = run_bass_kernel_spmd(prog, in_maps, core_ids=list(range(NCORES)),
                               trace=_trace, tmpdir=_tmpdir)
    out = assemble_output(res.results)
    if _trace:
        return out, res
    return out


# revision 13
# speedup vs baseline: 1.0211x; 1.0211x over previous
"""Trainium2 Bass kernel: batched single-head self-attention.

Reference computation (per (b, l) pair, 20 independent blocks):
    X = x[b, l] viewed as [N=1024, D=256] (xf layout)
    out[b, l] = softmax(beta * X @ X.T, axis=-1) @ X

Device algorithm (per block):
  * Scores: S[m, n] = sum_d X^T[d, m] X^T[d, n] on the TensorEngine with
    D on partitions. S is symmetric, so the PSUM tile doubles as the
    [keys, queries] orientation the second matmul wants.
  * Softmax shift: W[m, n] = exp(beta * (S[m, n] - c_n)) with
    c_n = ||x_n||^2. The per-QUERY shift rides the score matmul as one
    extra K=1 accumulation term (lhsT = ones row, rhs = -c row).
  * Second matmul: O^T[d, n] = sum_m xfo[m, d] W[m, n] with the value
    operand xfo = [X | 1 | 0] stationary. The [1|0] chunk gives the
    softmax denominator Z_n as a separate accumulation pass.
    Normalization (divide by Z) and the [d, n] -> [n, d] flip happen on
    the host.
  * Dtypes: score operands fp16 (10-bit mantissa; measured end-to-end
    rel err ~3e-3 vs the 2e-2 gate), W tiles and values bf16 (W must be
    bf16 not fp16: exp(S - c_n) reaches e^60, past fp16 range). 16-bit
    weights let every LDWEIGHTS use the fast weight-load path -- fp32
    weight loads (~224 ns per 128x128, no FWL) were the cadence limiter
    of the fp32r version (~275 ns per 512-col matmul vs 213 ns ideal).
  * Software pipelining: the AV matmuls of key tile a are emitted after
    the score matmuls of tile a+1, so ScalarE's exp of tile a runs under
    the tile-a+1 score streams and the PE never waits on ACT.
  * The Z pass of block s is emitted between the first and second score
    tiles of block s+1 so the PSUM evacuation it depends on overlaps
    score streaming instead of stalling the PE.

Sharding: 20 blocks over 8 cores as 2 full blocks + 1 half block (512
queries) per core. The half blocks use a host-side rotation of the key
axis so every core runs the identical program (softmax is invariant to
key permutation when values are permuted identically).
"""

import numpy as np
import ml_dtypes

import concourse.tile as tile
from concourse import bacc, mybir
from concourse.bass_utils import run_bass_kernel_spmd

F32 = mybir.dt.float32
F16 = mybir.dt.float16
BF16 = mybir.dt.bfloat16

B, L, D, H, W = 4, 5, 256, 32, 32
N = H * W            # 1024 keys per block
NBLK = B * L         # 20
NCORES = 8
NFULL = 2            # full blocks per core
NSLAB = 3            # 2 full + 1 half
DF = 272             # value operand row: [x(256) | 1 | 0...] padded so bf16
                     # rows are 544 B = 17x32 B (32 B-aligned weight rows)

EXP = mybir.ActivationFunctionType.Exp


def build_program(beta: float, fast: bool = True):
    sdt = F16 if fast else F32    # score operand dtype
    wdt = BF16 if fast else F32   # W tiles / value operand dtype
    # bf16, not fp16: the UNNORMALIZED O^T rows reach ~e^60 on contested
    # softmax rows (W = exp(S - c_n) can exceed fp16 range before the
    # host-side divide by Z)
    odt = BF16 if fast else F32   # output dtype
    nc = bacc.Bacc("TRN2", target_bir_lowering=False, debug=False,
                   num_devices=NCORES)
    # Inputs are host-packed in device layout so every DMA is a plain
    # contiguous transfer with large descriptors.
    xb_in = nc.dram_tensor("xb_in", [NSLAB, 2, 128, N], sdt,
                           kind="ExternalInput")
    xf_in = nc.dram_tensor("xf_in", [NSLAB, 128, 8, DF], wdt,
                           kind="ExternalInput")
    nc_in = nc.dram_tensor("nc_in", [1, NSLAB * N], sdt, kind="ExternalInput")
    yt_out = nc.dram_tensor("yt_out", [NSLAB, 128, 2, N], odt,
                            kind="ExternalOutput")
    z_out = nc.dram_tensor("z_out", [NSLAB, N], F32, kind="ExternalOutput")

    with tile.TileContext(nc) as tc:
        _build(tc, nc, xb_in.ap(), xf_in.ap(), nc_in.ap(), yt_out.ap(),
               z_out.ap(), beta, sdt, wdt, odt)
    nc.finalize()
    return nc


def _build(tc, nc, xb_in, xf_in, nc_in, yt_out, z_out, beta, sdt, wdt, odt):
    import contextlib
    ctx = contextlib.ExitStack()
    with ctx:
        const = ctx.enter_context(tc.tile_pool(name="const", bufs=1))
        xb_pool = ctx.enter_context(tc.tile_pool(name="xb", bufs=NSLAB))
        xfo_pool = ctx.enter_context(tc.tile_pool(name="xfo", bufs=NSLAB))
        negc_pool = ctx.enter_context(tc.tile_pool(name="negc", bufs=NSLAB))
        # W tiles stay live until the Z pass at the end of the block.
        w_pool = ctx.enter_context(tc.tile_pool(name="w", bufs=10))
        ot_sb_pool = ctx.enter_context(tc.tile_pool(name="ot_sb", bufs=2))
        z_sb_pool = ctx.enter_context(tc.tile_pool(name="z_sb", bufs=2))
        # PSUM: 2 score slots x 2 banks + 4 O^T accumulator banks = 8.
        # The Z-row accumulators reuse the O^T slots (same tag) after the
        # key loop, once the block's accumulators are evacuated.
        ps_s = ctx.enter_context(tc.tile_pool(name="ps_s", bufs=2, space="PSUM"))
        ps_od = ctx.enter_context(tc.tile_pool(name="ps_od", bufs=4, space="PSUM"))

        # Shift-rider stationary: a K=128 full-array operand with
        # partition 0 all-ones, rest zero, so lhsT.T @ negcb broadcasts
        # partition 0 of the moving operand to all 128 output rows.
        # A K=1 ones-row works too but a 1x128 (row-group-masked) matmul
        # forces a full PE drain before the next full-array LDWEIGHTS --
        # measured ~430 ns stall per rider. Emitted first on GpSimd so
        # the warmup matmuls (which use e1) can start immediately.
        e1 = const.tile([128, 128], sdt)
        nc.gpsimd.memset(e1[:], 0.0)
        nc.gpsimd.memset(e1[0:1, :], 1.0)

        # negcb: shift rows on partition 0, partitions 1..127 zeroed (they
        # meet zero weights in the rider, but NaN garbage x 0 = NaN).
        # Memset split across DVE and GpSimd so it finishes before the
        # first rider needs it.
        negcb = negc_pool.tile([128, NSLAB * N], sdt, tag="negc")
        half = NSLAB * N // 2
        nc.vector.memset(negcb[:, 0:half], 0.0)
        nc.gpsimd.memset(negcb[:, half:], 0.0)
        negcs = [negcb[:, s * N:(s + 1) * N] for s in range(NSLAB)]

        # Warm the PE clock (HAM) with throwaway matmuls that run during
        # the input-DMA window -- otherwise the first ~3.4us of real
        # matmuls run at half clock. Short 128-col streams keep the PE
        # queue shallow so the first real matmul isn't delayed.
        warm_ps = ps_od.tile([128, 512], F32, tag="od", name="warm_ps")
        for wi in range(6):
            nc.tensor.matmul(warm_ps[:, 0:128], e1[:], e1[:],
                             start=True, stop=True)

        # All input DMAs upfront, spread over queues so slab 0 lands
        # first: sync takes xb s0 chunk 0 then slabs 1-2; scalar takes
        # xb s0 chunk 1, the shift rows, then the value operands.
        xbs, xfos = [], []
        for s in range(NSLAB):
            xb = xb_pool.tile([128, 2, N], sdt, tag="xb", name=f"xb_{s}")
            xbs.append(xb)
        nc.sync.dma_start(out=xbs[0][:, 0, :], in_=xb_in[0][0])
        nc.scalar.dma_start(out=xbs[0][:, 1, :], in_=xb_in[0][1])
        for s in (1, 2):
            nc.sync.dma_start(
                out=xbs[s][:], in_=xb_in[s].rearrange("c p n -> p c n"))
        nc.scalar.dma_start(out=negcb[0:1, 0:half], in_=nc_in[:, 0:half])
        nc.scalar.dma_start(out=negcb[0:1, half:], in_=nc_in[:, half:])
        for s in range(NSLAB):
            xfo = xfo_pool.tile([128, 8, DF], wdt, tag="xfo",
                                name=f"xfo_{s}")
            nc.scalar.dma_start(out=xfo[:], in_=xf_in[s])
            xfos.append(xfo)

        def emit_scores(s, a, n_h):
            """Score matmuls + shift rider + exp for key tile a of slab s.
            Returns the W tile."""
            xb, negc = xbs[s], negcs[s]
            asl = slice(a * 128, (a + 1) * 128)
            sps = ps_s.tile([128, N], F32, tag="sps", name=f"sps_{s}_{a}")
            for c in range(2):
                for h in range(n_h):
                    hs = slice(h * 512, (h + 1) * 512)
                    nc.tensor.matmul(sps[:, hs], xb[:, c, asl], xb[:, c, hs],
                                     start=(c == 0), stop=False)
            for h in range(n_h):
                hs = slice(h * 512, (h + 1) * 512)
                nc.tensor.matmul(sps[:, hs], e1[:], negc[:, hs],
                                 start=False, stop=True)
            # W = exp(beta * S'), one ACT pass per key tile: two per-half
            # passes cost 2x686 ns = 1.37 us of ScalarE, which exceeds the
            # 1.28 us score window of the next tile and stalls the AV
            # matmuls; a single [128, n_q] pass is 1.11 us and fits.
            wt = w_pool.tile([128, N], wdt, tag="w", name=f"w_{s}_{a}")
            nc.scalar.activation(wt[:, 0:n_h * 512], sps[:, 0:n_h * 512],
                                 EXP, scale=float(beta))
            return wt

        def emit_av(s, a, n_h, od, wt):
            """O^T += xfo[a].T @ W[a] (value operand stationary)."""
            xfo = xfos[s]
            for ci, csl in ((0, slice(0, 128)), (1, slice(128, 256))):
                for h in range(n_h):
                    hs = slice(h * 512, (h + 1) * 512)
                    nc.tensor.matmul(od[ci][h][:], xfo[:, a, csl],
                                     wt[:, hs],
                                     start=(a == 0), stop=(a == 7))

        # Emission order (PE program order), 1-deep pipelined:
        #   scores(s,0) scores(s,1) av(s,0) scores(s,2) av(s,1) ...
        #   scores(s,7) av(s,6) av(s,7) zpass(s) scores(s+1,0) ...
        # The Z-row accumulator lives in a score-bank slot (free after
        # exp of the block's last tile), so the Z matmuls start right
        # after av(s,7) while the O^T evacuation runs on DVE/ACT in
        # parallel; the next block's AV then finds its banks free.
        for s in range(NSLAB):
            n_q = N if s < NFULL else N // 2
            n_h = n_q // 512
            od = None
            w_tiles = []
            for a in range(8):
                wt = emit_scores(s, a, n_h)
                w_tiles.append(wt)
                if a >= 1:
                    if od is None:
                        od = [[ps_od.tile([128, 512], F32, tag="od",
                                          name=f"od_{s}_{ci}_{h}")
                               for h in range(n_h)] for ci in range(2)]
                    emit_av(s, a - 1, n_h, od, w_tiles[a - 1])
            emit_av(s, 7, n_h, od, w_tiles[7])

            # Z[n] = sum_m W[m, n] via the [1|0] chunk of xfo, into a
            # score-bank slot.
            xfo = xfos[s]
            oz = ps_s.tile([128, N], F32, tag="sps", name=f"oz_{s}")
            for a in range(8):
                for h in range(n_h):
                    hs = slice(h * 512, (h + 1) * 512)
                    nc.tensor.matmul(oz[0:2, hs], xfo[:, a, 256:258],
                                     w_tiles[a][:, hs],
                                     start=(a == 0), stop=(a == 7))

            # Evacuate O^T accumulators on DVE/ACT, overlapping the Z
            # matmuls on the PE.
            ot_sb = ot_sb_pool.tile([128, 2, N], odt, tag="ot_sb",
                                    name=f"ot_{s}")
            z_sb = z_sb_pool.tile([1, N], F32, tag="z_sb", name=f"z_{s}")
            for h in range(n_h):
                hs = slice(h * 512, (h + 1) * 512)
                nc.vector.tensor_copy(ot_sb[:, 0, hs], od[0][h][:])
                nc.scalar.copy(ot_sb[:, 1, hs], od[1][h][:])
            for h in range(n_h):
                hs = slice(h * 512, (h + 1) * 512)
                # split across DVE and ACT so the copies run in parallel
                # and release the Z bank sooner
                if h == 0:
                    nc.vector.tensor_copy(z_sb[:, hs], oz[0:1, hs])
                else:
                    nc.scalar.copy(z_sb[:, hs], oz[0:1, hs])
            nc.sync.dma_start(out=yt_out[s], in_=ot_sb[:])
            nc.sync.dma_start(out=z_out[s][:n_q].unsqueeze(0),
                              in_=z_sb[:, :n_q])


_PROG_CACHE = {}


def _get_program(beta: float, fast: bool = True):
    key = (beta, fast)
    if key not in _PROG_CACHE:
        _PROG_CACHE[key] = build_program(beta, fast)
    return _PROG_CACHE[key]


def make_in_maps(x: np.ndarray, fast: bool = True):
    """Shard the full input [B, L, D, H, W] into 8 per-core input maps."""
    sdt = np.float16 if fast else np.float32
    wdt = ml_dtypes.bfloat16 if fast else np.float32
    xt_all = np.ascontiguousarray(x.reshape(NBLK, D, N))
    in_maps = []
    for c in range(NCORES):
        half_blk = NFULL * NCORES + c // 2
        half = xt_all[half_blk]
        if c % 2 == 1:
            # rotate keys so this core's queries are columns 0..511
            half = np.concatenate([half[:, N // 2:], half[:, :N // 2]], axis=1)
        slabs = np.stack([xt_all[NFULL * c], xt_all[NFULL * c + 1], half])
        slabs16 = slabs.astype(sdt)
        # shift row from the rounded operands (any per-query shift cancels
        # exactly in O/Z; using the rounded data keeps the overflow margin)
        s32 = slabs16.astype(np.float32)
        negc = -np.einsum('sdn,sdn->sn', s32, s32)
        xf = np.zeros((NSLAB, N, DF), np.float32)
        xf[:, :, :D] = slabs.transpose(0, 2, 1)
        xf[:, :, D] = 1.0
        # pack into device layout: xb [2, 128, N], xf [128, 8, DF]
        xb_p = slabs16.reshape(NSLAB, 2, 128, N)
        xf_p = np.ascontiguousarray(
            xf.reshape(NSLAB, 8, 128, DF).transpose(0, 2, 1, 3)).astype(wdt)
        in_maps.append({"xb_in": np.ascontiguousarray(xb_p),
                        "xf_in": xf_p,
                        "nc_in": np.ascontiguousarray(
                            negc.reshape(1, NSLAB * N)).astype(sdt)})
    return in_maps


def assemble_output(results):
    """Normalize, transpose and gather per-core outputs into [B, L, N, D]."""
    out = np.empty((NBLK, N, D), np.float32)
    for c in range(NCORES):
        # yt [NSLAB, 128, 2, N]: partition p, value chunk ci -> O^T row
        # ci*128 + p
        yt = np.asarray(results[c]["yt_out"], dtype=np.float32)
        yt = yt.transpose(0, 2, 1, 3).reshape(NSLAB, 2 * 128, N)
        z = results[c]["z_out"]
        for s, blk, lo, n_q in ((0, NFULL * c, 0, N),
                                (1, NFULL * c + 1, 0, N),
                                (2, NFULL * NCORES + c // 2,
                                 (c % 2) * (N // 2), N // 2)):
            ot = yt[s, :, :n_q]                       # [D, n_q], unnormalized
            out[blk, lo:lo + n_q] = (ot / z[s, :n_q]).T
    return out.reshape(B, L, N, D)


def kernel(x, beta, _trace=False, _fast=True, _tmpdir=None):
    x = np.asarray(x, dtype=np.float32)
    assert x.shape == (B, L, D, H, W), x.shape
    beta_f = float(np.asarray(beta))
    prog = _get_program(beta_f, _fast)
    in_maps = make_in_maps(x, _fast)
    res = run_bass_kernel_spmd(prog, in_maps, core_ids=list(range(NCORES)),
                               trace=_trace, tmpdir=_tmpdir)
    out = assemble_output(res.results)
    if _trace:
        return out, res
    return out
